# revision 2
# baseline (speedup 1.0000x reference)
"""GRU GenRNN Trainium2 kernel (nn_C_eAR_GenRNN).

Strategy: data-parallel over batch (B=32 -> 4 sequences per core, 8 cores).
Per core:
  Phase A: gates_x = [onehot(idx) | cnd] @ [G_tab | WcT]  (PE GEMM, bf16)
           where G_tab = emb @ W_emb.T + biases folded (host precompute).
           Output scaled by S_TOT = S_W*S_H so the recurrence psum (which
           accumulates S_W*S_H * W_hh@h) can absorb gx directly.
  Phase R: sequential GRU scan over T. W_hh@h runs in fp8e4m3 with
           perf_mode=DoubleRow (2 contraction-tiles per pass): weights
           scaled by S_W=256, h scaled by S_H=64 to stay in e4m3 normal
           range; descale folded into the sigmoid/tanh activation scale.
           Gate rows host-permuted into 2 groups [rz-block | n-block];
           gx added into PSUM via identity matmul (bf16, exact path).
  Phase C: hidden = relu(o_rnn @ fc1.T + b); out = hidden @ fc2.T + b (bf16).
Everything runs in one NEFF via TileContext; T-loop is a hardware For_i.
"""
import numpy as np
import ml_dtypes

import concourse.bass as bass
import concourse.bacc as bacc
import concourse.tile as tile
from concourse import mybir
from concourse.bass_utils import run_bass_kernel_spmd

BF16 = ml_dtypes.bfloat16
NP8 = ml_dtypes.float8_e4m3
F32 = mybir.dt.float32
BF = mybir.dt.bfloat16
FP8 = mybir.dt.float8e4

B, T_FULL, H, EMB, C2, O, FCD = 32, 4096, 896, 256, 512, 512, 896
NCORES, BL = 8, 4
THREEH = 3 * H
NCH = H // 128   # 7 chunks of 128 hidden units
NPAIR = 4        # fp8 DoubleRow pairs: H padded 896 -> 1024 = 4 x 256

S_W = 256.0      # W_hh scale into fp8e4m3 normal range
S_H = 64.0       # h scale into fp8e4m3 normal range
S_TOT = S_W * S_H
INV_S = 1.0 / S_TOT

# groups of chunks: [rz-block | n-block] each; sized so each fits 3 PSUM banks
GROUPS = [list(range(0, 4)), list(range(4, 7))]

Sig = mybir.ActivationFunctionType.Sigmoid
Tanh = mybir.ActivationFunctionType.Tanh
Relu = mybir.ActivationFunctionType.Relu
Ident = mybir.ActivationFunctionType.Identity
DR = mybir.MatmulPerfMode.DoubleRow


def _perm_and_groups():
    """Row permutation of the 3H gate dim into grouped [rz | n] layout."""
    perm = []
    ginfo = []  # (col_base, chunks, rzw, nw)
    base = 0
    for chunks in GROUPS:
        for k in chunks:
            perm.extend(range(k * 128, (k + 1) * 128))          # r_k
            perm.extend(range(H + k * 128, H + (k + 1) * 128))  # z_k
        for k in chunks:
            perm.extend(range(2 * H + k * 128, 2 * H + (k + 1) * 128))  # n_k
        rzw = len(chunks) * 256
        nw = len(chunks) * 128
        ginfo.append((base, chunks, rzw, nw))
        base += rzw + nw
    return np.array(perm), ginfo


PERM, GINFO = _perm_and_groups()


def _mm_windows(width):
    """Bank-aligned (<=512, non-straddling) windows covering [0, width)."""
    out = []
    pos = 0
    while pos < width:
        end = min(width, (pos // 512 + 1) * 512)
        out.append((pos, end - pos))
        pos = end
    return out


def build_nc(T=T_FULL, unroll=8):
    RT = T * BL
    nc = bacc.Bacc("TRN2", target_bir_lowering=False, debug=False,
                   num_devices=NCORES)
    dt = mybir.dt
    ein = lambda n, s, d: nc.dram_tensor(n, s, d, kind="ExternalInput").ap()
    idxb_d = ein("idxb", [128, RT], F32)
    cndr_d = ein("cndr", [RT, C2], F32)
    wstack_d = ein("wstack", [1024, THREEH], BF)
    whhp_d = ein("whhp", [NPAIR * 128, 2 * THREEH], FP8)
    fc1wt_d = ein("fc1wt", [H, FCD], BF)
    fc2wt_d = ein("fc2wt", [FCD, O], BF)
    fc1bt_d = ein("fc1bt", [128, FCD // 128], F32)
    fc2bt_d = ein("fc2bt", [128, O // 128], F32)
    eye4f_d = ein("eye4f", [BL, BL], F32)
    eye416_d = ein("eye416", [BL, 16], BF)
    eye128f_d = ein("eye128f", [128, 128], F32)
    gx_d = nc.dram_tensor("gx_i", [RT, THREEH], BF).ap()
    h_d = nc.dram_tensor("h_i", [RT, H], F32).ap()
    out_d = nc.dram_tensor("outp", [RT, O], F32, kind="ExternalOutput").ap()

    with tile.TileContext(nc) as tc:
        # ---------------- Phase A: gates_x GEMM ----------------
        with (
            tc.tile_pool(name="wA", bufs=1) as wA,
            tc.tile_pool(name="pa", bufs=3) as pa,
            tc.tile_pool(name="cstA", bufs=1) as cstA,
            tc.tile_pool(name="psA", bufs=6, space="PSUM") as psA,
            tc.tile_pool(name="psTA", bufs=2, space="PSUM") as psTA,
        ):
            eye128f = cstA.tile([128, 128], F32)
            nc.sync.dma_start(eye128f[:], eye128f_d[:])
            wstack_sb = []
            for k in range(8):
                wt = wA.tile([128, THREEH], BF, tag=f"wst{k}", name=f"wst{k}")
                nc.sync.dma_start(wt[:], wstack_d[k * 128:(k + 1) * 128, :])
                wstack_sb.append(wt)
            it32 = cstA.tile([128, 1], mybir.dt.int32)
            nc.gpsimd.iota(it32[:], pattern=[[1, 1]], base=0, channel_multiplier=1)
            ocs = []
            for k in range(4):
                oc = cstA.tile([128, 1], F32, tag=f"oc{k}", name=f"oc{k}")
                nc.vector.tensor_scalar_add(oc[:], it32[:], float(128 * k))
                ocs.append(oc)

            wins = _mm_windows(THREEH)
            for rt in range(RT // 128):
                idxt = pa.tile([128, 128], F32)
                nc.sync.dma_start(idxt[:], idxb_d[:, rt * 128:(rt + 1) * 128])
                xT = []
                for k in range(4):
                    oh = pa.tile([128, 128], BF, tag=f"oh{k}", name=f"oh{k}")
                    nc.vector.tensor_scalar(oh[:], idxt[:], ocs[k][:], None,
                                            op0=mybir.AluOpType.is_equal)
                    xT.append(oh)
                for k in range(4):
                    ct = pa.tile([128, 128], F32, tag=f"ct{k}", name=f"ct{k}")
                    nc.sync.dma_start(
                        ct[:], cndr_d[rt * 128:(rt + 1) * 128,
                                      k * 128:(k + 1) * 128])
                    pst = psTA.tile([128, 128], F32)
                    nc.tensor.transpose(pst[:], ct[:], eye128f[:])
                    cb = pa.tile([128, 128], BF, tag=f"cb{k}", name=f"cb{k}")
                    nc.vector.tensor_copy(cb[:], pst[:])
                    xT.append(cb)
                gxall = pa.tile([128, THREEH], BF, tag="gxall", name="gxall")
                for (w0, wl) in wins:
                    pg = psA.tile([128, 512], F32, tag="pgA", name="pgA")
                    for k in range(8):
                        nc.tensor.matmul(pg[:, :wl], xT[k][:],
                                         wstack_sb[k][:, w0:w0 + wl],
                                         start=(k == 0), stop=(k == 7))
                    nc.vector.tensor_copy(gxall[:, w0:w0 + wl], pg[:, :wl])
                nc.sync.dma_start(gx_d[rt * 128:(rt + 1) * 128, :], gxall[:])

        # ---------------- Phase R: GRU recurrence ----------------
        with (
            tc.tile_pool(name="wR", bufs=1) as wR,
            tc.tile_pool(name="stR", bufs=1) as stR,
            tc.tile_pool(name="pr", bufs=3) as pr,
            tc.tile_pool(name="psR", bufs=1, space="PSUM") as psR,
            tc.tile_pool(name="psTR", bufs=2, space="PSUM") as psTR,
        ):
            whhp_sb = []
            for p in range(NPAIR):
                wt = wR.tile([128, 2, THREEH], FP8, tag=f"whp{p}", name=f"whp{p}")
                nc.sync.dma_start(wt[:], whhp_d[p * 128:(p + 1) * 128, :])
                whhp_sb.append(wt)
            eye4f = wR.tile([BL, BL], F32, tag="eye4f")
            nc.sync.dma_start(eye4f[:], eye4f_d[:])
            eye416 = wR.tile([BL, 16], BF, tag="eye416")
            nc.sync.dma_start(eye416[:], eye416_d[:])
            h_sb = stR.tile([BL, H], F32, tag="h")
            nc.vector.memset(h_sb[:], 0.0)
            # hT pairs: [128, pair, sub, 16] fp8; batch in slots 0..3, rest 0
            hT_sb = stR.tile([128, NPAIR, 2, 16], FP8, tag="hT")
            nc.vector.memset(hT_sb[:], 0.0)

            def step(row0):
                gxb = pr.tile([BL, THREEH], BF, tag="gxb", name="gxb")
                nc.sync.dma_start(gxb[:], gx_d[row0, :])
                pgs = []
                for gi, (gb, chunks, rzw, nw) in enumerate(GINFO):
                    pgs.append(psR.tile([16, rzw + nw], F32, tag=f"pg{gi}",
                                        name=f"pg{gi}"))
                # PE: pair-tiles outer so low-k work is available early
                for p in range(NPAIR):
                    for gi, (gb, chunks, rzw, nw) in enumerate(GINFO):
                        for (w0, wl) in _mm_windows(rzw + nw):
                            nc.tensor.matmul(
                                pgs[gi][:, w0:w0 + wl], hT_sb[:, p, :, :],
                                whhp_sb[p][:, :, gb + w0:gb + w0 + wl],
                                start=(p == 0), stop=(p == NPAIR - 1),
                                perf_mode=DR)
                    if p == 0:
                        for gi, (gb, chunks, rzw, nw) in enumerate(GINFO):
                            for (w0, wl) in _mm_windows(rzw):
                                nc.tensor.matmul(
                                    pgs[gi][:, w0:w0 + wl], eye416[:],
                                    gxb[:, gb + w0:gb + w0 + wl],
                                    start=False, stop=False)
                for gi, (gb, chunks, rzw, nw) in enumerate(GINFO):
                    ch = len(chunks)
                    pg = pgs[gi]
                    rzs = pr.tile([BL, rzw], BF, tag=f"rzs{gi}", name=f"rzs{gi}")
                    nc.scalar.activation(rzs[:], pg[0:BL, 0:rzw], Sig,
                                         scale=INV_S)
                    rAP = rzs[:, 0:rzw].rearrange("p (c two k) -> p c two k",
                                                  two=2, k=128)[:, :, 0, :]
                    zAP = rzs[:, 0:rzw].rearrange("p (c two k) -> p c two k",
                                                  two=2, k=128)[:, :, 1, :]
                    nAPp = pg[0:BL, rzw:rzw + nw].rearrange("p (c k) -> p c k",
                                                            k=128)
                    rnt = pr.tile([BL, ch, 128], F32, tag=f"rnt{gi}", name=f"rnt{gi}")
                    nc.vector.tensor_mul(rnt[:], rAP, nAPp)
                    gxn = gxb[:, gb + rzw:gb + rzw + nw].rearrange(
                        "p (c k) -> p c k", k=128)
                    nnt = pr.tile([BL, ch, 128], F32, tag=f"nnt{gi}", name=f"nnt{gi}")
                    nc.vector.tensor_add(nnt[:], rnt[:], gxn)
                    nb = pr.tile([BL, ch, 128], BF, tag=f"nb{gi}", name=f"nb{gi}")
                    nc.scalar.activation(nb[:], nnt[:], Tanh, scale=INV_S)
                    hsl = h_sb[:, chunks[0] * 128:(chunks[-1] + 1) * 128]
                    hAP = hsl.rearrange("p (c k) -> p c k", k=128)
                    zht = pr.tile([BL, ch, 128], F32, tag=f"zht{gi}", name=f"zht{gi}")
                    nc.vector.tensor_mul(zht[:], zAP, hAP)
                    omz = pr.tile([BL, ch, 128], BF, tag=f"omz{gi}", name=f"omz{gi}")
                    nc.vector.tensor_scalar(omz[:], zAP, -1.0, 1.0,
                                            op0=mybir.AluOpType.mult,
                                            op1=mybir.AluOpType.add)
                    t1 = pr.tile([BL, ch, 128], BF, tag=f"t1{gi}", name=f"t1{gi}")
                    nc.vector.tensor_mul(t1[:], omz[:], nb[:])
                    nc.vector.tensor_add(hAP, t1[:], zht[:])
                    for k in chunks:
                        pt = psTR.tile([128, BL], F32, tag="pt", name="pt")
                        nc.tensor.transpose(
                            pt[:], h_sb[:, k * 128:(k + 1) * 128], eye4f[:])
                        nc.vector.tensor_scalar_mul(
                            hT_sb[:, k // 2, k % 2, 0:BL], pt[:], S_H)
                nc.sync.dma_start(h_d[row0, :], h_sb[:])

            with tc.For_i(0, RT, BL * unroll) as ivr:
                for u in range(unroll):
                    step(bass.ds(ivr + BL * u, BL))

        # ---------------- Phase C: FC layers ----------------
        with (
            tc.tile_pool(name="wC", bufs=1) as wC,
            tc.tile_pool(name="pcp", bufs=2) as pcp,
            tc.tile_pool(name="psC1", bufs=2, space="PSUM") as psC1,
            tc.tile_pool(name="psC2", bufs=2, space="PSUM") as psC2,
            tc.tile_pool(name="psTC", bufs=2, space="PSUM") as psTC,
        ):
            eye128fc = wC.tile([128, 128], F32, tag="eye128fc")
            nc.sync.dma_start(eye128fc[:], eye128f_d[:])
            fc1w_sb, fc2w_sb = [], []
            for k in range(NCH):
                wt = wC.tile([128, FCD], BF, tag=f"fc1w{k}", name=f"fc1w{k}")
                nc.sync.dma_start(wt[:], fc1wt_d[k * 128:(k + 1) * 128, :])
                fc1w_sb.append(wt)
                wt2 = wC.tile([128, O], BF, tag=f"fc2w{k}", name=f"fc2w{k}")
                nc.sync.dma_start(wt2[:], fc2wt_d[k * 128:(k + 1) * 128, :])
                fc2w_sb.append(wt2)
            fc1b_sb = wC.tile([128, FCD // 128], F32, tag="fc1b")
            nc.sync.dma_start(fc1b_sb[:], fc1bt_d[:])
            fc2b_sb = wC.tile([128, O // 128], F32, tag="fc2b")
            nc.sync.dma_start(fc2b_sb[:], fc2bt_d[:])

            n_rc = RT // 512
            for rc in range(n_rc):
                r0 = rc * 512
                oT = []
                for fi in range(NCH):
                    ot = pcp.tile([128, 512], BF, tag=f"oT{fi}", name=f"oT{fi}")
                    for ri in range(4):
                        ht = pcp.tile([128, 128], F32, tag="htC", name="htC")
                        nc.sync.dma_start(
                            ht[:], h_d[r0 + ri * 128:r0 + (ri + 1) * 128,
                                       fi * 128:(fi + 1) * 128])
                        ps = psTC.tile([128, 128], F32, tag="ptC", name="ptC")
                        nc.tensor.transpose(ps[:], ht[:], eye128fc[:])
                        nc.vector.tensor_copy(ot[:, ri * 128:(ri + 1) * 128],
                                              ps[:])
                    oT.append(ot)
                hid = []
                for mi in range(NCH):
                    h1 = psC1.tile([128, 512], F32, tag="h1", name="h1")
                    for ki in range(NCH):
                        nc.tensor.matmul(
                            h1[:], fc1w_sb[ki][:, mi * 128:(mi + 1) * 128],
                            oT[ki][:], start=(ki == 0), stop=(ki == NCH - 1))
                    hd = pcp.tile([128, 512], BF, tag=f"hid{mi}", name=f"hid{mi}")
                    nc.scalar.activation(hd[:], h1[:], Relu,
                                         bias=fc1b_sb[:, mi:mi + 1])
                    hid.append(hd)
                orows = [pcp.tile([128, O], F32, tag=f"orow{ri}", name=f"orow{ri}")
                         for ri in range(4)]
                for oi in range(O // 128):
                    o2 = psC2.tile([128, 512], F32, tag="o2", name="o2")
                    for ki in range(NCH):
                        nc.tensor.matmul(
                            o2[:], fc2w_sb[ki][:, oi * 128:(oi + 1) * 128],
                            hid[ki][:], start=(ki == 0), stop=(ki == NCH - 1))
                    ob = pcp.tile([128, 512], F32, tag="obC", name="obC")
                    nc.scalar.activation(ob[:], o2[:], Ident,
                                         bias=fc2b_sb[:, oi:oi + 1])
                    for ri in range(4):
                        ps = psTC.tile([128, 128], F32, tag="ptC", name="ptC")
                        nc.tensor.transpose(ps[:],
                                            ob[:, ri * 128:(ri + 1) * 128],
                                            eye128fc[:])
                        nc.vector.tensor_copy(
                            orows[ri][:, oi * 128:(oi + 1) * 128], ps[:])
                for ri in range(4):
                    nc.sync.dma_start(
                        out_d[r0 + ri * 128:r0 + (ri + 1) * 128, :],
                        orows[ri][:])

    nc.compile()
    return nc


_NC_CACHE = {}


def _host_prep(reference_sample, i_cnd_series, emb, w_ih, w_hh, b_ih, b_hh,
               fc1_w, fc1_b, fc2_w, fc2_b, T):
    w_ih = np.asarray(w_ih, np.float32)[PERM]
    w_hh = np.asarray(w_hh, np.float32)[PERM]
    b_ih = np.asarray(b_ih, np.float32)[PERM]
    b_hh = np.asarray(b_hh, np.float32)[PERM]
    # rz positions (within permuted layout) get b_hh folded into gx bias
    rz_mask = np.zeros(THREEH, np.float32)
    for (gb, chunks, rzw, nw) in GINFO:
        rz_mask[gb:gb + rzw] = 1.0
    bias_row = b_ih + b_hh * rz_mask
    if np.any(np.abs(b_hh * (1 - rz_mask)) > 0):
        raise NotImplementedError("nonzero b_hh n-gate not supported")
    # gx scaled by S_TOT so it can add directly into the scaled psum
    G_tab = ((np.asarray(emb, np.float32) @ w_ih[:, :EMB].T)
             + bias_row[None, :]) * S_TOT
    WcT = w_ih[:, EMB:].T.copy() * S_TOT
    wstack = np.concatenate([G_tab, WcT], 0).astype(BF16)
    # W_hh^T scaled, padded 896->1024, pair layout [p*128+q, i*3H+n]
    whT = w_hh.T.copy() * S_W                      # [H, 3H]
    whT = np.concatenate([whT, np.zeros((NPAIR * 256 - H, THREEH),
                                        np.float32)], 0)
    whhp = (whT.reshape(NPAIR, 2, 128, THREEH)
            .transpose(0, 2, 1, 3).reshape(NPAIR * 128, 2 * THREEH)
            .astype(NP8))
    fc1wt = np.asarray(fc1_w, np.float32).T.copy().astype(BF16)
    fc2wt = np.asarray(fc2_w, np.float32).T.copy().astype(BF16)
    fc1bt = np.asarray(fc1_b, np.float32).reshape(FCD // 128, 128).T.copy()
    fc2bt = np.asarray(fc2_b, np.float32).reshape(O // 128, 128).T.copy()
    eye4f = np.eye(BL, dtype=np.float32)
    eye416 = np.zeros((BL, 16), np.float32)
    eye416[:, :BL] = np.eye(BL)
    eye416 = eye416.astype(BF16)
    eye128f = np.eye(128, dtype=np.float32)
    shared = dict(wstack=wstack, whhp=whhp, fc1wt=fc1wt, fc2wt=fc2wt,
                  fc1bt=fc1bt, fc2bt=fc2bt, eye4f=eye4f, eye416=eye416,
                  eye128f=eye128f)
    sample = np.asarray(reference_sample)
    cnd = np.asarray(i_cnd_series, np.float32)
    in_maps = []
    for c in range(NCORES):
        sl = slice(c * BL, (c + 1) * BL)
        idx = sample[sl, :T].T.reshape(-1).astype(np.float32)  # (T*BL,)
        idxb = np.broadcast_to(idx[None, :], (128, T * BL)).copy()
        cndr = np.ascontiguousarray(
            cnd[sl, :T].transpose(1, 0, 2)).reshape(T * BL, C2)
        in_maps.append(dict(idxb=idxb, cndr=cndr, **shared))
    return in_maps


def kernel(reference_sample, i_cnd_series, emb, w_ih, w_hh, b_ih, b_hh,
           fc1_w, fc1_b, fc2_w, fc2_b, T=None, unroll=8):
    T = T or np.asarray(reference_sample).shape[1]
    in_maps = _host_prep(reference_sample, i_cnd_series, emb, w_ih, w_hh,
                         b_ih, b_hh, fc1_w, fc1_b, fc2_w, fc2_b, T)
    key = (T, unroll)
    if key not in _NC_CACHE:
        _NC_CACHE[key] = build_nc(T, unroll)
    nc = _NC_CACHE[key]
    res = run_bass_kernel_spmd(nc, in_maps, core_ids=list(range(NCORES)))
    outs = []
    for c in range(NCORES):
        o = res.results[c]["outp"].reshape(T, BL, O).transpose(1, 0, 2)
        outs.append(o)
    return np.concatenate(outs, 0).astype(np.float32)


# revision 9
# speedup vs baseline: 52.9320x; 52.9320x over previous
"""GRU GenRNN Trainium2 kernel (nn_C_eAR_GenRNN).

Strategy: data-parallel over batch (B=32 -> 4 sequences per core, 8 cores).
Per core:
  Phase A: gates_x = [onehot(idx) | cnd] @ [G_tab | WcT]  (PE GEMM, bf16)
           where G_tab = emb @ W_emb.T + biases folded (host precompute).
           Output scaled by S_TOT = S_W*S_H so the recurrence psum (which
           accumulates S_W*S_H * W_hh@h) can absorb gx directly.
  Phase R: sequential GRU scan over T. W_hh@h runs in fp8e4m3 with
           perf_mode=DoubleRow (2 contraction-tiles per pass): weights
           scaled by S_W=256, h scaled by S_H=64 to stay in e4m3 normal
           range; descale folded into the sigmoid/tanh activation scale.
           Gate rows host-permuted into 2 groups [rz-block | n-block];
           gx added into PSUM via identity matmul (bf16, exact path).
  Phase C: hidden = relu(o_rnn @ fc1.T + b); out = hidden @ fc2.T + b (bf16).
Everything runs in one NEFF via TileContext; T-loop is a hardware For_i.
"""
import numpy as np
import ml_dtypes

import concourse.bass as bass
import concourse.bacc as bacc
import concourse.tile as tile
from concourse import mybir
from concourse.bass_utils import run_bass_kernel_spmd

BF16 = ml_dtypes.bfloat16
NP8 = ml_dtypes.float8_e4m3
F32 = mybir.dt.float32
BF = mybir.dt.bfloat16
FP8 = mybir.dt.float8e4

B, T_FULL, H, EMB, C2, O, FCD = 32, 4096, 896, 256, 512, 512, 896
NCORES, BL = 8, 4
THREEH = 3 * H
NCH = H // 128   # 7 chunks of 128 hidden units
NPAIR = 4        # fp8 DoubleRow pairs: H padded 896 -> 1024 = 4 x 256

S_W = 256.0      # W_hh scale into fp8e4m3 normal range
S_H = 64.0       # h scale into fp8e4m3 normal range
S_TOT = S_W * S_H
INV_S = 1.0 / S_TOT

# single group: plain [r | z | n] layout, one PSUM tile for all gates
GROUPS = [list(range(0, 7))]

Sig = mybir.ActivationFunctionType.Sigmoid
Tanh = mybir.ActivationFunctionType.Tanh
Relu = mybir.ActivationFunctionType.Relu
Ident = mybir.ActivationFunctionType.Identity
DR = mybir.MatmulPerfMode.DoubleRow


def _perm_and_groups():
    """Row permutation of the 3H gate dim into grouped [rz | n] layout."""
    perm = []
    ginfo = []  # (col_base, chunks, rzw, nw)
    base = 0
    for chunks in GROUPS:
        for k in chunks:
            perm.extend(range(k * 128, (k + 1) * 128))          # r_k
            perm.extend(range(H + k * 128, H + (k + 1) * 128))  # z_k
        for k in chunks:
            perm.extend(range(2 * H + k * 128, 2 * H + (k + 1) * 128))  # n_k
        rzw = len(chunks) * 256
        nw = len(chunks) * 128
        ginfo.append((base, chunks, rzw, nw))
        base += rzw + nw
    return np.array(perm), ginfo


PERM, GINFO = _perm_and_groups()


def _mm_windows(width):
    """Bank-aligned (<=512, non-straddling) windows covering [0, width)."""
    out = []
    pos = 0
    while pos < width:
        end = min(width, (pos // 512 + 1) * 512)
        out.append((pos, end - pos))
        pos = end
    return out


def build_nc(T=T_FULL, unroll=8):
    RT = T * BL
    nc = bacc.Bacc("TRN2", target_bir_lowering=False, debug=False,
                   num_devices=NCORES)
    dt = mybir.dt
    ein = lambda n, s, d: nc.dram_tensor(n, s, d, kind="ExternalInput").ap()
    idxb_d = ein("idxb", [128, RT], F32)
    cndr_d = ein("cndr", [RT, C2], F32)
    wstack_d = ein("wstack", [1024, THREEH], BF)
    whhp_d = ein("whhp", [NPAIR * 128, 2 * THREEH], FP8)
    fc1wt_d = ein("fc1wt", [H, FCD], BF)
    fc2wt_d = ein("fc2wt", [FCD, O], BF)
    fc1bt_d = ein("fc1bt", [128, FCD // 128], F32)
    fc2bt_d = ein("fc2bt", [128, O // 128], F32)
    eye4f_d = ein("eye4f", [BL, BL], F32)
    eye416_d = ein("eye416", [BL, 16], BF)
    eye128f_d = ein("eye128f", [128, 128], F32)
    gx_d = nc.dram_tensor("gx_i", [RT, THREEH], BF).ap()
    h_d = nc.dram_tensor("h_i", [RT, H], F32).ap()
    out_d = nc.dram_tensor("outp", [RT, O], F32, kind="ExternalOutput").ap()

    with tile.TileContext(nc) as tc:
        # ---------------- Phase A: gates_x GEMM ----------------
        with (
            tc.tile_pool(name="wA", bufs=1) as wA,
            tc.tile_pool(name="pa", bufs=3) as pa,
            tc.tile_pool(name="cstA", bufs=1) as cstA,
            tc.tile_pool(name="psA", bufs=6, space="PSUM") as psA,
            tc.tile_pool(name="psTA", bufs=2, space="PSUM") as psTA,
        ):
            eye128f = cstA.tile([128, 128], F32)
            nc.sync.dma_start(eye128f[:], eye128f_d[:])
            wstack_sb = []
            for k in range(8):
                wt = wA.tile([128, THREEH], BF, tag=f"wst{k}", name=f"wst{k}")
                nc.sync.dma_start(wt[:], wstack_d[k * 128:(k + 1) * 128, :])
                wstack_sb.append(wt)
            it32 = cstA.tile([128, 1], mybir.dt.int32)
            nc.gpsimd.iota(it32[:], pattern=[[1, 1]], base=0, channel_multiplier=1)
            ocs = []
            for k in range(4):
                oc = cstA.tile([128, 1], F32, tag=f"oc{k}", name=f"oc{k}")
                nc.vector.tensor_scalar_add(oc[:], it32[:], float(128 * k))
                ocs.append(oc)

            wins = _mm_windows(THREEH)
            for rt in range(RT // 128):
                idxt = pa.tile([128, 128], F32)
                nc.sync.dma_start(idxt[:], idxb_d[:, rt * 128:(rt + 1) * 128])
                xT = []
                for k in range(4):
                    oh = pa.tile([128, 128], BF, tag=f"oh{k}", name=f"oh{k}")
                    nc.vector.tensor_scalar(oh[:], idxt[:], ocs[k][:], None,
                                            op0=mybir.AluOpType.is_equal)
                    xT.append(oh)
                for k in range(4):
                    ct = pa.tile([128, 128], F32, tag=f"ct{k}", name=f"ct{k}")
                    nc.sync.dma_start(
                        ct[:], cndr_d[rt * 128:(rt + 1) * 128,
                                      k * 128:(k + 1) * 128])
                    pst = psTA.tile([128, 128], F32)
                    nc.tensor.transpose(pst[:], ct[:], eye128f[:])
                    cb = pa.tile([128, 128], BF, tag=f"cb{k}", name=f"cb{k}")
                    nc.vector.tensor_copy(cb[:], pst[:])
                    xT.append(cb)
                gxall = pa.tile([128, THREEH], BF, tag="gxall", name="gxall")
                for (w0, wl) in wins:
                    pg = psA.tile([128, 512], F32, tag="pgA", name="pgA")
                    for k in range(8):
                        nc.tensor.matmul(pg[:, :wl], xT[k][:],
                                         wstack_sb[k][:, w0:w0 + wl],
                                         start=(k == 0), stop=(k == 7))
                    nc.vector.tensor_copy(gxall[:, w0:w0 + wl], pg[:, :wl])
                nc.sync.dma_start(gx_d[rt * 128:(rt + 1) * 128, :], gxall[:])

        # ---------------- Phase R: GRU recurrence ----------------
        with (
            tc.tile_pool(name="wR", bufs=1) as wR,
            tc.tile_pool(name="stR", bufs=1) as stR,
            tc.tile_pool(name="pr", bufs=4) as pr,
            tc.tile_pool(name="psR", bufs=1, space="PSUM") as psR,
            tc.tile_pool(name="psTR", bufs=1, space="PSUM") as psTR,
        ):
            whhp_sb = []
            for p in range(NPAIR):
                wt = wR.tile([128, 2, THREEH], FP8, tag=f"whp{p}", name=f"whp{p}")
                nc.sync.dma_start(wt[:], whhp_d[p * 128:(p + 1) * 128, :])
                whhp_sb.append(wt)
            eye4f = wR.tile([BL, BL], F32, tag="eye4f")
            nc.sync.dma_start(eye4f[:], eye4f_d[:])
            eye416 = wR.tile([BL, 16], BF, tag="eye416")
            nc.sync.dma_start(eye416[:], eye416_d[:])
            h_sb = stR.tile([BL, H], F32, tag="h")
            nc.vector.memset(h_sb[:], 0.0)
            # hT pairs: [128, pair, sub, 16] fp8; batch in slots 0..3, rest 0
            hT_sb = stR.tile([128, NPAIR, 2, 16], FP8, tag="hT")
            nc.vector.memset(hT_sb[:], 0.0)
            # one psum tile collects all 7 h-chunk transposes (slot 7 stays 0)
            ptall = psTR.tile([128, 2 * NPAIR, BL], F32, tag="ptall")
            nc.vector.memset(ptall[:], 0.0)

            def step(row0):
                gxb = pr.tile([BL, THREEH], BF, tag="gxb", name="gxb")
                nc.sync.dma_start(gxb[:], gx_d[row0, :])
                pgs = []
                for gi, (gb, chunks, rzw, nw) in enumerate(GINFO):
                    pgs.append(psR.tile([16, rzw + nw], F32, tag=f"pg{gi}",
                                        name=f"pg{gi}"))
                # PE: pair-tiles outer so low-k work is available early
                for p in range(NPAIR):
                    for gi, (gb, chunks, rzw, nw) in enumerate(GINFO):
                        for (w0, wl) in _mm_windows(rzw + nw):
                            nc.tensor.matmul(
                                pgs[gi][:, w0:w0 + wl], hT_sb[:, p, :, :],
                                whhp_sb[p][:, :, gb + w0:gb + w0 + wl],
                                start=(p == 0), stop=(p == NPAIR - 1),
                                perf_mode=DR)
                    if p == 0:
                        for gi, (gb, chunks, rzw, nw) in enumerate(GINFO):
                            for (w0, wl) in _mm_windows(rzw):
                                nc.tensor.matmul(
                                    pgs[gi][:, w0:w0 + wl], eye416[:],
                                    gxb[:, gb + w0:gb + w0 + wl],
                                    start=False, stop=False)
                for gi, (gb, chunks, rzw, nw) in enumerate(GINFO):
                    ch = len(chunks)
                    pg = pgs[gi]
                    rzs = pr.tile([BL, rzw], BF, tag=f"rzs{gi}", name=f"rzs{gi}")
                    nc.scalar.activation(rzs[:], pg[0:BL, 0:rzw], Sig,
                                         scale=INV_S)
                    rAP = rzs[:, 0:rzw].rearrange("p (c two k) -> p c two k",
                                                  two=2, k=128)[:, :, 0, :]
                    zAP = rzs[:, 0:rzw].rearrange("p (c two k) -> p c two k",
                                                  two=2, k=128)[:, :, 1, :]
                    nAPp = pg[0:BL, rzw:rzw + nw].rearrange("p (c k) -> p c k",
                                                            k=128)
                    rnt = pr.tile([BL, ch, 128], F32, tag=f"rnt{gi}", name=f"rnt{gi}")
                    nc.vector.tensor_mul(rnt[:], rAP, nAPp)
                    gxn = gxb[:, gb + rzw:gb + rzw + nw].rearrange(
                        "p (c k) -> p c k", k=128)
                    nnt = pr.tile([BL, ch, 128], F32, tag=f"nnt{gi}", name=f"nnt{gi}")
                    nc.vector.tensor_add(nnt[:], rnt[:], gxn)
                    nb = pr.tile([BL, ch, 128], BF, tag=f"nb{gi}", name=f"nb{gi}")
                    nc.scalar.activation(nb[:], nnt[:], Tanh, scale=INV_S)
                    hsl = h_sb[:, chunks[0] * 128:(chunks[-1] + 1) * 128]
                    hAP = hsl.rearrange("p (c k) -> p c k", k=128)
                    # h_new = n + z*(h - n)
                    dhn = pr.tile([BL, ch, 128], F32, tag=f"dhn{gi}", name=f"dhn{gi}")
                    nc.vector.tensor_sub(dhn[:], hAP, nb[:])
                    zd = pr.tile([BL, ch, 128], F32, tag=f"zd{gi}", name=f"zd{gi}")
                    nc.vector.tensor_mul(zd[:], zAP, dhn[:])
                    nc.vector.tensor_add(hAP, nb[:], zd[:])
                    for k in chunks:
                        nc.tensor.transpose(
                            ptall[:, k, :], h_sb[:, k * 128:(k + 1) * 128],
                            eye4f[:])
                    nc.vector.tensor_scalar_mul(
                        hT_sb[:, :, :, 0:BL],
                        ptall[:].rearrange("p (a b) c -> p a b c", b=2), S_H)
                nc.sync.dma_start(h_d[row0, :], h_sb[:])

            with tc.For_i(0, RT, BL * unroll) as ivr:
                for u in range(unroll):
                    step(bass.ds(ivr + BL * u, BL))

        # ---------------- Phase C: FC layers ----------------
        with (
            tc.tile_pool(name="wC", bufs=1) as wC,
            tc.tile_pool(name="pcp", bufs=2) as pcp,
            tc.tile_pool(name="psC1", bufs=2, space="PSUM") as psC1,
            tc.tile_pool(name="psC2", bufs=2, space="PSUM") as psC2,
            tc.tile_pool(name="psTC", bufs=2, space="PSUM") as psTC,
        ):
            eye128fc = wC.tile([128, 128], F32, tag="eye128fc")
            nc.sync.dma_start(eye128fc[:], eye128f_d[:])
            fc1w_sb, fc2w_sb = [], []
            for k in range(NCH):
                wt = wC.tile([128, FCD], BF, tag=f"fc1w{k}", name=f"fc1w{k}")
                nc.sync.dma_start(wt[:], fc1wt_d[k * 128:(k + 1) * 128, :])
                fc1w_sb.append(wt)
                wt2 = wC.tile([128, O], BF, tag=f"fc2w{k}", name=f"fc2w{k}")
                nc.sync.dma_start(wt2[:], fc2wt_d[k * 128:(k + 1) * 128, :])
                fc2w_sb.append(wt2)
            fc1b_sb = wC.tile([128, FCD // 128], F32, tag="fc1b")
            nc.sync.dma_start(fc1b_sb[:], fc1bt_d[:])
            fc2b_sb = wC.tile([128, O // 128], F32, tag="fc2b")
            nc.sync.dma_start(fc2b_sb[:], fc2bt_d[:])

            n_rc = RT // 512
            for rc in range(n_rc):
                r0 = rc * 512
                oT = []
                for fi in range(NCH):
                    ot = pcp.tile([128, 512], BF, tag=f"oT{fi}", name=f"oT{fi}")
                    for ri in range(4):
                        ht = pcp.tile([128, 128], F32, tag="htC", name="htC")
                        nc.sync.dma_start(
                            ht[:], h_d[r0 + ri * 128:r0 + (ri + 1) * 128,
                                       fi * 128:(fi + 1) * 128])
                        ps = psTC.tile([128, 128], F32, tag="ptC", name="ptC")
                        nc.tensor.transpose(ps[:], ht[:], eye128fc[:])
                        nc.vector.tensor_copy(ot[:, ri * 128:(ri + 1) * 128],
                                              ps[:])
                    oT.append(ot)
                hid = []
                for mi in range(NCH):
                    h1 = psC1.tile([128, 512], F32, tag="h1", name="h1")
                    for ki in range(NCH):
                        nc.tensor.matmul(
                            h1[:], fc1w_sb[ki][:, mi * 128:(mi + 1) * 128],
                            oT[ki][:], start=(ki == 0), stop=(ki == NCH - 1))
                    hd = pcp.tile([128, 512], BF, tag=f"hid{mi}", name=f"hid{mi}")
                    nc.scalar.activation(hd[:], h1[:], Relu,
                                         bias=fc1b_sb[:, mi:mi + 1])
                    hid.append(hd)
                orows = [pcp.tile([128, O], F32, tag=f"orow{ri}", name=f"orow{ri}")
                         for ri in range(4)]
                for oi in range(O // 128):
                    o2 = psC2.tile([128, 512], F32, tag="o2", name="o2")
                    for ki in range(NCH):
                        nc.tensor.matmul(
                            o2[:], fc2w_sb[ki][:, oi * 128:(oi + 1) * 128],
                            hid[ki][:], start=(ki == 0), stop=(ki == NCH - 1))
                    ob = pcp.tile([128, 512], F32, tag="obC", name="obC")
                    nc.scalar.activation(ob[:], o2[:], Ident,
                                         bias=fc2b_sb[:, oi:oi + 1])
                    for ri in range(4):
                        ps = psTC.tile([128, 128], F32, tag="ptC", name="ptC")
                        nc.tensor.transpose(ps[:],
                                            ob[:, ri * 128:(ri + 1) * 128],
                                            eye128fc[:])
                        nc.vector.tensor_copy(
                            orows[ri][:, oi * 128:(oi + 1) * 128], ps[:])
                for ri in range(4):
                    nc.sync.dma_start(
                        out_d[r0 + ri * 128:r0 + (ri + 1) * 128, :],
                        orows[ri][:])

    nc.compile()
    return nc


_NC_CACHE = {}


def _host_prep(reference_sample, i_cnd_series, emb, w_ih, w_hh, b_ih, b_hh,
               fc1_w, fc1_b, fc2_w, fc2_b, T):
    w_ih = np.asarray(w_ih, np.float32)[PERM]
    w_hh = np.asarray(w_hh, np.float32)[PERM]
    b_ih = np.asarray(b_ih, np.float32)[PERM]
    b_hh = np.asarray(b_hh, np.float32)[PERM]
    # rz positions (within permuted layout) get b_hh folded into gx bias
    rz_mask = np.zeros(THREEH, np.float32)
    for (gb, chunks, rzw, nw) in GINFO:
        rz_mask[gb:gb + rzw] = 1.0
    bias_row = b_ih + b_hh * rz_mask
    if np.any(np.abs(b_hh * (1 - rz_mask)) > 0):
        raise NotImplementedError("nonzero b_hh n-gate not supported")
    # gx scaled by S_TOT so it can add directly into the scaled psum
    G_tab = ((np.asarray(emb, np.float32) @ w_ih[:, :EMB].T)
             + bias_row[None, :]) * S_TOT
    WcT = w_ih[:, EMB:].T.copy() * S_TOT
    wstack = np.concatenate([G_tab, WcT], 0).astype(BF16)
    # W_hh^T scaled, padded 896->1024, pair layout [p*128+q, i*3H+n]
    whT = w_hh.T.copy() * S_W                      # [H, 3H]
    whT = np.concatenate([whT, np.zeros((NPAIR * 256 - H, THREEH),
                                        np.float32)], 0)
    whhp = (whT.reshape(NPAIR, 2, 128, THREEH)
            .transpose(0, 2, 1, 3).reshape(NPAIR * 128, 2 * THREEH)
            .astype(NP8))
    fc1wt = np.asarray(fc1_w, np.float32).T.copy().astype(BF16)
    fc2wt = np.asarray(fc2_w, np.float32).T.copy().astype(BF16)
    fc1bt = np.asarray(fc1_b, np.float32).reshape(FCD // 128, 128).T.copy()
    fc2bt = np.asarray(fc2_b, np.float32).reshape(O // 128, 128).T.copy()
    eye4f = np.eye(BL, dtype=np.float32)
    eye416 = np.zeros((BL, 16), np.float32)
    eye416[:, :BL] = np.eye(BL)
    eye416 = eye416.astype(BF16)
    eye128f = np.eye(128, dtype=np.float32)
    shared = dict(wstack=wstack, whhp=whhp, fc1wt=fc1wt, fc2wt=fc2wt,
                  fc1bt=fc1bt, fc2bt=fc2bt, eye4f=eye4f, eye416=eye416,
                  eye128f=eye128f)
    sample = np.asarray(reference_sample)
    cnd = np.asarray(i_cnd_series, np.float32)
    in_maps = []
    for c in range(NCORES):
        sl = slice(c * BL, (c + 1) * BL)
        idx = sample[sl, :T].T.reshape(-1).astype(np.float32)  # (T*BL,)
        idxb = np.broadcast_to(idx[None, :], (128, T * BL)).copy()
        cndr = np.ascontiguousarray(
            cnd[sl, :T].transpose(1, 0, 2)).reshape(T * BL, C2)
        in_maps.append(dict(idxb=idxb, cndr=cndr, **shared))
    return in_maps


def kernel(reference_sample, i_cnd_series, emb, w_ih, w_hh, b_ih, b_hh,
           fc1_w, fc1_b, fc2_w, fc2_b, T=None, unroll=8):
    T = T or np.asarray(reference_sample).shape[1]
    in_maps = _host_prep(reference_sample, i_cnd_series, emb, w_ih, w_hh,
                         b_ih, b_hh, fc1_w, fc1_b, fc2_w, fc2_b, T)
    key = (T, unroll)
    if key not in _NC_CACHE:
        _NC_CACHE[key] = build_nc(T, unroll)
    nc = _NC_CACHE[key]
    res = run_bass_kernel_spmd(nc, in_maps, core_ids=list(range(NCORES)))
    outs = []
    for c in range(NCORES):
        o = res.results[c]["outp"].reshape(T, BL, O).transpose(1, 0, 2)
        outs.append(o)
    return np.concatenate(outs, 0).astype(np.float32)
